# revision 1
# baseline (speedup 1.0000x reference)
"""Trainium2 Bass kernel for nn_CrossAttention_44693429682227.

Math (reference):
    q = (x @ Wq.T) / E**0.25, reshaped (b, t, H, E)
    scores = q @ keys.T over a shared bank of N=50000 (key, scalar-value) pairs
    attn = softmax(scores, axis=-1)
    out = mean_h(attn @ values) + curiosity  -> (b, t, 1)

Because values are scalars, out_row = (sum_n exp(s_n) * v_n) / (sum_n exp(s_n)).
Scores are bounded (|s| <~ 20), so f32 exp never overflows and no max-
subtraction is needed; numerator and denominator partials are exact to merge
across key-bank shards.

Distribution: the key bank is sharded 8 ways (49 blocks of 128 = 6272
keys/core, 50176 padded globally); every core computes the projection for all
4096 (b,t,h) query rows (replicated, cheap) and full partial num/den sums over
its shard. Host merges partials.

Engine budget per core (the kernel is ACT-bound; everything else must hide
under the ~200us of exp work):
  ACT:  8 heads x 17 exp instructions (16x [128,1536] + 1x [128,512])  ~200us
  PE :  scores 392 mm + nd-reduce on (49-KOFF) blocks/head + projection ~190us
  DVE:  nd-reduce on KOFF blocks/head via scalar_tensor_tensor into f32
        accumulators, folded back into the PSUM accumulation by two
        ones-column matmuls per head                                   ~130us

kernel.py is self-contained: shapes/sharding hardcoded, no sibling imports.
"""

import os
import sys
from contextlib import ExitStack

import numpy as np

if "/opt/trn_rl_repo" not in sys.path:
    sys.path.insert(0, "/opt/trn_rl_repo")

# Problem shapes (hardcoded per contract)
B, T = 4, 128
BT = B * T            # 512 query (b,t) rows
HIN = 1024
H, E = 8, 128
N = 50000
NCORES = 8

# Sharding / tiling
NBLK = 49             # key-blocks (128 keys each) per core
KC = NBLK * 128       # 6272 keys per core
NPAD = KC * NCORES    # 50176 padded bank size
KCH = HIN // 128      # 8 contraction chunks for the projection
GROUPS = [3] * 16 + [1]   # ACT exp group sizes (PSUM: 2 pools x 3 banks)

KOFF = int(os.environ.get("KOFF", "0"))    # blocks/head reduced on DVE
TRACE = bool(int(os.environ.get("KTRACE", "0")))

# Evenly spread offloaded blocks across 1..47 (0 and 48 stay on PE so the
# PSUM accumulation group starts/ends on a plain nd matmul).
if KOFF > 0:
    OFFLOAD = sorted({1 + round(i * 46 / max(KOFF - 1, 1)) for i in range(KOFF)})
else:
    OFFLOAD = []
assert 0 not in OFFLOAD and 48 not in OFFLOAD and len(OFFLOAD) == KOFF

LAST_RESULTS = None   # BassKernelResults of the most recent run (for test.py)

_cache = {}


def _install_ntff_hook():
    """Register the axon NTFF profile hook that this image's antenv lacks.

    bass_utils reads it via ``antenv.axon_hooks.get_axon_ntff_profile_hook``;
    we synthesize that module around trn_agent_boot's ctypes implementation.
    Also soften ``upload_artifacts`` (no bucket access needed for local runs).
    """
    import types

    if "antenv.axon_hooks" in sys.modules:
        return
    try:
        from trn_agent_boot.trn_boot import _ntff_profile_via_ctypes

        hook = _ntff_profile_via_ctypes("/opt/axon/libaxon_pjrt.so")
    except Exception:
        hook = None
    mod = types.ModuleType("antenv.axon_hooks")
    mod.get_axon_ntff_profile_hook = lambda: hook
    sys.modules["antenv.axon_hooks"] = mod

    from concourse import bass_utils as bu

    orig_upload = bu.upload_artifacts

    def safe_upload(tmpdir):
        try:
            return orig_upload(tmpdir)
        except Exception as e:
            return f"upload-skipped ({type(e).__name__})"

    bu.upload_artifacts = safe_upload

    bu.upload_artifacts = safe_upload


def _build():
    import concourse.tile as tile
    from concourse import bacc, mybir

    f32 = mybir.dt.float32
    f32r = mybir.dt.float32r
    bf16 = mybir.dt.bfloat16
    Alu = mybir.AluOpType
    Exp = mybir.ActivationFunctionType.Exp

    nc = bacc.Bacc(trn_type="TRN2", target_bir_lowering=False, debug=False)

    # Host pre-arranges xt/wqt so every DMA is one contiguous run per
    # partition: xt[p, k, bt] = x[bt, 128k+p]; wqt[h, p, k, e] = Wq.T[128k+p, 128h+e]
    c0_d = nc.dram_tensor("c0", [128, 2], f32r, kind="ExternalInput")
    xt_d = nc.dram_tensor("xt", [128, KCH * BT], bf16, kind="ExternalInput")
    wqt_d = nc.dram_tensor("wqt", [H, 128, KCH * E], bf16, kind="ExternalInput")
    keyst_d = nc.dram_tensor("keyst", [E, KC], bf16, kind="ExternalInput")
    vb_d = nc.dram_tensor("vb", [128, NBLK * 3], bf16, kind="ExternalInput")
    vsc_d = nc.dram_tensor("vsc", [128, NBLK * 2], f32, kind="ExternalInput")
    nd_d = nc.dram_tensor("nd_out", [3, H * BT], f32, kind="ExternalOutput")

    with tile.TileContext(nc) as tc, ExitStack() as ctx:
        singles = ctx.enter_context(tc.tile_pool(name="singles", bufs=1))
        epool = ctx.enter_context(tc.tile_pool(name="epool", bufs=4))
        apool = ctx.enter_context(tc.tile_pool(name="apool", bufs=2))
        ps_s = ctx.enter_context(tc.tile_pool(name="ps_s", bufs=2, space="PSUM"))
        ps_q = ctx.enter_context(tc.tile_pool(name="ps_q", bufs=1, space="PSUM"))
        ps_sm = ctx.enter_context(tc.tile_pool(name="ps_sm", bufs=1, space="PSUM"))

        # ---- persistent SBUF loads, critical-path first (HWDGE drains FIFO)
        def load(name, shape, src, dt=bf16):
            t = singles.tile(shape, dt, name=name, tag=name)
            nc.sync.dma_start(out=t, in_=src)
            return t

        c0_sb = load("c0", [128, 2], c0_d.ap(), f32r)
        xt = load("xt", [128, KCH, BT], xt_d.ap().rearrange("p (k b) -> p k b", b=BT))
        wq_h = [None] * H
        keyst_c = [None] * len(GROUPS)

        def load_wq(h):
            wq_h[h] = load(
                f"wq{h}", [128, KCH, E],
                wqt_d.ap()[h].rearrange("p (k e) -> p k e", e=E),
            )

        goff = [0]
        for gb in GROUPS:
            goff.append(goff[-1] + gb)

        def load_kc(g):
            lo, hi = goff[g] * 128, goff[g + 1] * 128
            keyst_c[g] = load(f"keyst{g}", [128, hi - lo], keyst_d.ap()[:, lo:hi])

        load_wq(0)
        load_kc(0)
        load_kc(1)
        load_wq(1)
        for g in range(2, 5):
            load_kc(g)
        vb_sb = load(
            "vb", [128, NBLK, 3], vb_d.ap().rearrange("p (b c) -> p b c", c=3)
        )
        vsc_sb = load(
            "vsc", [128, NBLK, 2],
            vsc_d.ap().rearrange("p (b c) -> p b c", c=2), f32,
        ) if OFFLOAD else None
        load_wq(2)
        for g in range(5, 8):
            load_kc(g)
        load_wq(3)
        for g in range(8, 11):
            load_kc(g)
        load_wq(4)
        for g in range(11, 14):
            load_kc(g)
        load_wq(5)
        for g in range(14, len(GROUPS)):
            load_kc(g)
        load_wq(6)
        load_wq(7)

        qt_sb = singles.tile([128, H, BT], bf16, name="qt")
        out_sb = singles.tile([3, H, BT], f32, name="out")
        warm = singles.tile([128, 1], f32, name="warm")

        # Pull the exp table load off the critical path: first ACTIVATE on a
        # new set costs ~2.7us; run it while DMAs land.
        nc.scalar.activation(warm, c0_sb[:, 0:1], Exp)

        ones_ap = c0_sb[:, 1:2]

        # ---- projection: q_ps = sum_k wq[h][:,k,:]^T @ xt[:,k,:]
        # proj(0) runs up front; later heads are spread one matmul per group
        # inside the main loop so PE never starves ACT.
        def proj_mm(h, k):
            q_ps = proj_mm.cur
            if q_ps is None or proj_mm.head != h:
                q_ps = ps_q.tile([128, BT], f32, tag="q", name=f"q_ps{h}")
                proj_mm.cur, proj_mm.head = q_ps, h
            nc.tensor.matmul(
                q_ps, lhsT=wq_h[h][:, k, :], rhs=xt[:, k, :],
                start=(k == 0), stop=(k == KCH - 1),
            )
            if k == KCH - 1:
                nc.vector.tensor_copy(qt_sb[:, h, :], q_ps)

        proj_mm.cur, proj_mm.head = None, None

        for k in range(KCH):
            proj_mm(0, k)

        # proj schedule: head 0 carries proj(1) in groups 1-8 and proj(2) in
        # groups 9-16; head h>=1 carries proj(h+2) in groups 2-9.
        def proj_slot(h, g):
            if h == 0:
                if 1 <= g <= 8:
                    return (1, g - 1)
                if 9 <= g <= 16:
                    return (2, g - 9)
            elif h <= 5 and 2 <= g <= 9:
                return (h + 2, g - 2)
            return None

        # ---- main loop: per head, 17 score/exp groups + nd reduction
        for h in range(H):
            nd_ps = ps_sm.tile([3, BT], f32, tag="sm", name=f"nd_ps{h}")
            accn = apool.tile([128, BT], f32r, tag="accn", name=f"accn{h}")
            accd = apool.tile([128, BT], f32r, tag="accd", name=f"accd{h}")
            first_off = OFFLOAD[0] if OFFLOAD else -1
            for g, gb in enumerate(GROUPS):
                s_ps = ps_s.tile([128, 3, BT], f32, tag="s", name=f"s_ps_{h}_{g}")
                for j in range(gb):
                    b = goff[g] + j
                    nc.tensor.matmul(
                        s_ps[:, j, :],
                        lhsT=keyst_c[g][:, 128 * j:128 * (j + 1)],
                        rhs=qt_sb[:, h, :],
                        start=True,
                        stop=True,
                    )
                eT = epool.tile([128, 3, BT], bf16, tag="e", name=f"eT_{h}_{g}")
                nc.scalar.activation(eT[:, 0:gb, :], s_ps[:, 0:gb, :], Exp)
                for j in range(gb):
                    b = goff[g] + j
                    if b in OFFLOAD:
                        v_ap = vsc_sb[:, b, 0:1]
                        m_ap = vsc_sb[:, b, 1:2]
                        if b == first_off:
                            nc.vector.tensor_scalar(
                                accn, eT[:, j, :], v_ap, None, Alu.mult
                            )
                            nc.vector.tensor_scalar(
                                accd, eT[:, j, :], m_ap, None, Alu.mult
                            )
                        else:
                            nc.vector.scalar_tensor_tensor(
                                accn, eT[:, j, :], v_ap, accn,
                                op0=Alu.mult, op1=Alu.add,
                            )
                            nc.vector.scalar_tensor_tensor(
                                accd, eT[:, j, :], m_ap, accd,
                                op0=Alu.mult, op1=Alu.add,
                            )
                    else:
                        nc.tensor.matmul(
                            nd_ps,
                            lhsT=vb_sb[:, b, :],
                            rhs=eT[:, j, :],
                            start=(b == 0),
                            stop=(not OFFLOAD and b == NBLK - 1),
                        )
                slot = proj_slot(h, g)
                if slot is not None:
                    proj_mm(*slot)
            if OFFLOAD:
                nc.tensor.matmul(
                    nd_ps[0:1, :], lhsT=ones_ap, rhs=accn,
                    start=False, stop=False, skip_group_check=True,
                )
                nc.tensor.matmul(
                    nd_ps[2:3, :], lhsT=ones_ap, rhs=accd,
                    start=False, stop=True, skip_group_check=True,
                )
            nc.vector.tensor_copy(out_sb[:, h, :], nd_ps)

        nc.sync.dma_start(out=nd_d.ap(), in_=out_sb.rearrange("p h b -> p (h b)"))

    nc.compile()
    return nc


def _prep_inputs(x, Wq, keys, values):
    import ml_dtypes

    f32 = np.float32
    bf = ml_dtypes.bfloat16

    # xt[p, k, bt] = x[bt, 128k+p]  (one contiguous run per partition)
    xT = np.ascontiguousarray(
        np.asarray(x, dtype=f32).reshape(BT, KCH, 128).transpose(2, 1, 0)
    ).reshape(128, KCH * BT).astype(bf)
    # wqt[h, p, k, e] = Wq.T[128k+p, 128h+e], with 1/E**0.25 folded in
    wq_s = np.asarray(Wq, dtype=f32) * np.float32(E ** -0.25)  # [oc, hin]
    wqT = np.ascontiguousarray(
        wq_s.reshape(H, E, KCH, 128).transpose(0, 3, 2, 1)  # [h, p, k, e]
    ).reshape(H, 128, KCH * E).astype(bf)

    keys_pad = np.zeros((NPAD, E), dtype=f32)
    keys_pad[:N] = np.asarray(keys, dtype=f32)
    keysT = np.ascontiguousarray(keys_pad.T).astype(bf)  # [E, NPAD]

    v_pad = np.zeros(NPAD, dtype=f32)
    v_pad[:N] = np.asarray(values, dtype=f32)
    mask = np.zeros(NPAD, dtype=f32)
    mask[:N] = 1.0
    v_hi = v_pad.astype(bf).astype(f32)
    v_lo = v_pad - v_hi

    # vb[core][p, blk, 3] with p = key index within 128-block
    def shard_cols(a):  # [NPAD] -> [NCORES, 128, NBLK]
        return a.reshape(NCORES, NBLK, 128).transpose(0, 2, 1)

    vb = np.stack([shard_cols(v_hi), shard_cols(v_lo), shard_cols(mask)], axis=-1)
    vb = np.ascontiguousarray(vb).astype(bf)  # [NCORES, 128, NBLK, 3]
    vsc = np.stack([shard_cols(v_pad), shard_cols(mask)], axis=-1)
    vsc = np.ascontiguousarray(vsc).astype(f32)  # [NCORES, 128, NBLK, 2]

    c0 = np.ones((128, 2), dtype=f32)

    in_maps = []
    for c in range(NCORES):
        in_maps.append(
            {
                "c0": c0,
                "xt": xT,
                "wqt": wqT,
                "keyst": np.ascontiguousarray(keysT[:, c * KC:(c + 1) * KC]),
                "vb": np.ascontiguousarray(vb[c].reshape(128, NBLK * 3)),
                "vsc": np.ascontiguousarray(vsc[c].reshape(128, NBLK * 2)),
            }
        )
    return in_maps


def kernel(x, curiosity_score, Wq, keys, values):
    global LAST_RESULTS
    if TRACE:
        _install_ntff_hook()
    from concourse.bass_utils import run_bass_kernel_spmd

    if "nc" not in _cache:
        _cache["nc"] = _build()
    nc = _cache["nc"]

    in_maps = _prep_inputs(x, Wq, keys, values)

    res = run_bass_kernel_spmd(
        nc, in_maps, core_ids=list(range(NCORES)), trace=TRACE
    )
    LAST_RESULTS = res

    nd = np.stack(
        [np.asarray(res.results[c]["nd_out"], dtype=np.float64) for c in range(NCORES)]
    ).reshape(NCORES, 3, H, BT)
    num = (nd[:, 0] + nd[:, 1]).sum(axis=0)  # [H, BT]
    den = nd[:, 2].sum(axis=0)               # [H, BT]
    out = (num / den).mean(axis=0) + np.asarray(
        curiosity_score, dtype=np.float64
    ).reshape(BT)
    return out.astype(np.float32).reshape(B, T, 1)

